# revision 16
# baseline (speedup 1.0000x reference)
"""NetVLAD layer on 8 Trainium2 NeuronCores (Bass/Tile), v4.

Problem: descriptors [B=16, D=512, N=4096] f32, W [K=64, D], b [K],
centers [D, K].
  scores = softmax_K(W @ desc + b)            [B, K, N]
  agg[b,d,k] = sum_n scores[b,k,n] desc[b,d,n]
  vlad = agg - centers * sum_n(scores);  intra-L2-norm over D; global L2.

Sharding: data-parallel over B across 8 cores (2 items per core);
W/b/centers replicated.

v4 design (v1 121.6us -> v2 113.5 -> v3 103.6us):
  v3 was dependency/latency bound: per 4-chunk group PE stalled on the
  softmax chain, ACT paid a separate accumulator-readout per chunk, and
  Exp<->Sqrt activation-table reloads cost 1.3us each.  v4 restructures
  for throughput:
  - desc pre-cast fp8e4m3, both layouts, as in v3 (8.4 MB/core DMA).
  - mm1 keeps W stationary (tiny LDWEIGHTS) and streams desc via fp8
    DoubleRow, producing scores [2x64k, 512n] packed on 128 partitions
    (half 0 at partitions 0-63 via quadrant (0,0), half 1 at 64-127 via
    quadrant (0,64)).  One ACT exp per strip with the natural
    per-partition bias ([b;b]), full 128 lanes.
  - exp chunks are PE-transposed (bf16, identity quadrants) into a
    [128, 8, K] PSUM group; softmax is 3 DVE ops per strip: 3D reduce
    -> Z, reciprocal, and one broadcast multiply -> softT fp8.
  - mm2: agg[K, D] via 4 fp8 DoubleRow matmuls per strip (2 n-chunks
    each); ssum via DoubleRow ones-column-sums, both items packed in
    one PSUM bank at partition offsets 0/64.
  - the two batch items are interleaved strip-by-strip, transposes run
    one stage behind mm1 and mm2 two stages behind, so PE always has
    independent work while a strip's softmax chain completes.
  - global L2 norm folded to the exact constant 1/sqrt(K)=0.125 (after
    intra-norm every k-column has unit norm), so the tail needs a
    single Sqrt per item and only one activation-table switch, after
    all exps are done.
"""

import sys

sys.path.insert(0, "/opt/trn_rl_repo")

import numpy as np
import ml_dtypes

B, D, K, N = 16, 512, 64, 4096
N_CORES = 8
B_PER = B // N_CORES           # 2 items per core
DT = D // 128                  # 4 d-tiles
S = 4                          # n-strips per item
NSTR = N // S                  # 1024 columns per strip
CPS = NSTR // 128              # 8 n-chunks of 128 per strip

_CACHE = {}


def _build():
    import concourse.bass as bass  # noqa: F401
    import concourse.tile as tile
    from concourse import bacc, mybir
    from contextlib import ExitStack

    bf16 = mybir.dt.bfloat16
    f8 = mybir.dt.float8e4
    f32 = mybir.dt.float32
    AF = mybir.ActivationFunctionType
    OP = mybir.AluOpType
    AX = mybir.AxisListType
    DR = mybir.MatmulPerfMode.DoubleRow

    nc = bacc.Bacc("TRN2", target_bir_lowering=False, debug=False,
                   num_devices=N_CORES)

    # desc strips, [item, strip, partition, ...]: one 4 KB row per partition
    da_d = nc.dram_tensor("da", [B_PER, S, 128, DT, NSTR], f8,
                          kind="ExternalInput").ap()
    dt_d = nc.dram_tensor("dt", [B_PER, S, 128, CPS, 512], f8,
                          kind="ExternalInput").ap()
    wt_d = nc.dram_tensor("wt", [DT, 128, K], f8, kind="ExternalInput").ap()
    b_d = nc.dram_tensor("bias", [64, 1], f32, kind="ExternalInput").ap()
    eye_d = nc.dram_tensor("eye", [64, 64], bf16,
                           kind="ExternalInput").ap()
    ones2_d = nc.dram_tensor("ones2", [128, 2, 1], f8,
                             kind="ExternalInput").ap()
    cneg_d = nc.dram_tensor("cneg", [K, D], f32, kind="ExternalInput").ap()
    out_d = nc.dram_tensor("out", [B_PER, K, D], f32,
                           kind="ExternalOutput").ap()

    with tile.TileContext(nc) as tc, ExitStack() as ctx:
        const = ctx.enter_context(tc.tile_pool(name="const", bufs=1))
        sdesc = ctx.enter_context(tc.tile_pool(name="sdesc", bufs=3))
        sdt = ctx.enter_context(tc.tile_pool(name="sdt", bufs=4))
        pexp = ctx.enter_context(tc.tile_pool(name="pexp", bufs=3))
        psoft = ctx.enter_context(tc.tile_pool(name="psoft", bufs=3))
        small = ctx.enter_context(tc.tile_pool(name="small", bufs=16))
        med = ctx.enter_context(tc.tile_pool(name="med", bufs=2))
        # PSUM bank budget (8): scA 1 + scB 1 + xt 2 + agg 2 + ssA 1 + ssB 1
        ps_sc = ctx.enter_context(tc.tile_pool(name="ps_sc", bufs=1,
                                               space="PSUM"))
        ps_xt = ctx.enter_context(tc.tile_pool(name="ps_xt", bufs=2,
                                               space="PSUM"))
        ps_agg = ctx.enter_context(tc.tile_pool(name="ps_agg", bufs=2,
                                                space="PSUM"))
        ps_ss = ctx.enter_context(tc.tile_pool(name="ps_ss", bufs=2,
                                               space="PSUM"))

        # ---- constants (scalar HWDGE queue) ----
        wt_sb = const.tile([128, DT, K], f8, tag="wt")
        for t in range(DT):
            nc.scalar.dma_start(out=wt_sb[:, t, :], in_=wt_d[t])
        b_sb = const.tile([64, 1], f32, tag="b")
        nc.scalar.dma_start(out=b_sb[:], in_=b_d[:])
        eye_sb = const.tile([64, 64], bf16, tag="eye")
        nc.scalar.dma_start(out=eye_sb[:], in_=eye_d[:])
        ones2_sb = const.tile([128, 2, 1], f8, tag="ones2")
        nc.scalar.dma_start(out=ones2_sb[:], in_=ones2_d[:])
        cneg_sb = const.tile([K, D], f32, tag="cneg")
        nc.scalar.dma_start(out=cneg_sb[:], in_=cneg_d[:])
        eps_sb = const.tile([K, 1], f32, tag="eps")
        nc.vector.memset(eps_sb[:], 1e-24)

        agg_tiles = [ps_agg.tile([K, D], f32, tag="agg", name=f"agg{i}")
                     for i in range(B_PER)]
        ss_tiles = [ps_ss.tile([K, 1], f32, tag="ss", name=f"ss{i}")
                    for i in range(B_PER)]

        pend_tr = []   # (i, s, exp_s, dTt) awaiting transpose+softmax
        pend_mm2 = []  # (i, s, soft_g, dTt) awaiting mm2

        def emit_tr(grp):
            i, s, exps, dTt = grp
            xt = ps_xt.tile([128, CPS, K], bf16, tag="xt",
                            name=f"xt{i}{s}")
            for c in range(CPS):
                h, cc = divmod(c, 4)
                nc.tensor.transpose(
                    xt[:, c, :],
                    exps[h][:, 128 * cc:128 * (cc + 1)],
                    eye_sb[:],
                )
            z8 = small.tile([128, CPS], f32, tag="z", name=f"z{i}{s}")
            nc.vector.reduce_sum(z8[:], xt[:], axis=AX.X)
            r8 = small.tile([128, CPS], f32, tag="r", name=f"r{i}{s}")
            nc.vector.reciprocal(r8[:], z8[:])
            soft_g = psoft.tile([128, CPS, K], f8, tag="soft",
                                name=f"soft{i}{s}")
            nc.vector.tensor_mul(soft_g[:], xt[:],
                                 r8[:, :, None].broadcast_to((128, CPS, K)))
            pend_mm2.append((i, s, soft_g, dTt))

        def emit_mm2(grp):
            i, s, soft_g, dTt = grp
            for j in range(CPS // 2):
                nc.tensor.matmul(
                    agg_tiles[i][:], lhsT=soft_g[:, 2 * j:2 * j + 2, :],
                    rhs=dTt[:, 2 * j:2 * j + 2, :], perf_mode=DR,
                    start=(s == 0 and j == 0),
                    stop=(s == S - 1 and j == CPS // 2 - 1))
            for j in range(CPS // 2):
                nc.tensor.matmul(
                    ss_tiles[i][:], lhsT=soft_g[:, 2 * j:2 * j + 2, :],
                    rhs=ones2_sb[:], perf_mode=DR,
                    start=(s == 0 and j == 0),
                    stop=(s == S - 1 and j == CPS // 2 - 1))

        for s in range(S):
            for i in range(B_PER):
                dbf = sdesc.tile([128, DT, NSTR], f8, tag="dbf",
                                 name=f"dbf{i}{s}")
                nc.sync.dma_start(out=dbf[:], in_=da_d[i, s])
                dTt = sdt.tile([128, CPS, 512], f8, tag="dT",
                               name=f"dT{i}{s}")
                nc.scalar.dma_start(out=dTt[:], in_=dt_d[i, s])
                # mm1: scores halves [64k, 512n] via fp8 DoubleRow,
                # W stationary; exp(scores + b) -> bf16 per half
                exps = []
                for h in range(2):
                    scp = ps_sc.tile([64, 512], f32, tag=f"sc{h}",
                                     name=f"sc{h}_{i}{s}")
                    for T in range(2):
                        nc.tensor.matmul(
                            scp[:],
                            lhsT=wt_sb[:, 2 * T:2 * T + 2, :],
                            rhs=dbf[:, 2 * T:2 * T + 2,
                                    512 * h:512 * (h + 1)],
                            perf_mode=DR, start=(T == 0), stop=(T == 1))
                    exp_h = pexp.tile([64, 512], bf16, tag=f"exps{h}",
                                      name=f"exps{h}_{i}{s}")
                    nc.scalar.activation(out=exp_h[:], in_=scp[:],
                                         func=AF.Exp, bias=b_sb[:],
                                         scale=1.0)
                    exps.append(exp_h)
                pend_tr.append((i, s, exps, dTt))
                # software pipeline: transposes 1 stage behind, mm2 2 behind
                if len(pend_tr) > 1:
                    emit_tr(pend_tr.pop(0))
                if len(pend_mm2) > 1:
                    emit_mm2(pend_mm2.pop(0))
        def emit_tails():
            # both items interleaved, ACT funcs grouped to minimize
            # activation-table switches (Square in exp table; Ln+Exp pair)
            vlads, sss, rns, outs = {}, {}, {}, {}
            for i in range(B_PER):
                ssum_sb = small.tile([K, 1], f32, tag="ssum",
                                     name=f"ssum{i}")
                nc.scalar.copy(ssum_sb[:], ss_tiles[i][:])
                vlad_sb = med.tile([K, D], f32, tag="vlad",
                                   name=f"vlad{i}")
                nc.vector.scalar_tensor_tensor(
                    vlad_sb[:], in0=cneg_sb[:], scalar=ssum_sb[:],
                    in1=agg_tiles[i][:], op0=OP.mult, op1=OP.add,
                )
                vlads[i] = vlad_sb
            for i in range(B_PER):
                sq_sb = med.tile([K, D], f32, tag="sq", name=f"sq{i}")
                ss_sb = small.tile([K, 1], f32, tag="ss2", name=f"ss2{i}")
                nc.scalar.activation(sq_sb[:], vlads[i][:], func=AF.Square,
                                     accum_out=ss_sb[:])
                sss[i] = ss_sb
            lns = {}
            for i in range(B_PER):
                ln_sb = small.tile([K, 1], f32, tag="ln", name=f"ln{i}")
                nc.scalar.activation(ln_sb[:], sss[i][:], func=AF.Ln)
                lns[i] = ln_sb
            for i in range(B_PER):
                rn_sb = small.tile([K, 1], f32, tag="rn", name=f"rn{i}")
                nc.scalar.activation(rn_sb[:], lns[i][:], func=AF.Exp,
                                     scale=-0.5)
                rns[i] = rn_sb
            for i in range(B_PER):
                # intra-norm by rn; global norm is exactly 1/sqrt(K)=0.125
                outT_sb = med.tile([K, D], f32, tag="outT",
                                   name=f"outT{i}")
                nc.vector.tensor_scalar(out=outT_sb[:], in0=vlads[i][:],
                                        scalar1=rns[i][:], scalar2=0.125,
                                        op0=OP.mult, op1=OP.mult)
                nc.sync.dma_start(out=out_d[i], in_=outT_sb[:])

        while pend_tr:
            emit_tr(pend_tr.pop(0))
        while pend_mm2:
            emit_mm2(pend_mm2.pop(0))
        emit_tails()

    nc.compile()
    return nc


def _get_nc():
    if "nc" not in _CACHE:
        _CACHE["nc"] = _build()
    return _CACHE["nc"]


def _host_inputs(descriptors, W, b, centers):
    f8 = ml_dtypes.float8_e4m3fn
    d16 = np.asarray(descriptors, dtype=np.float32).astype(f8)  # [B, D, N]
    wt = np.ascontiguousarray(
        W.astype(np.float32).T.reshape(DT, 128, K)).astype(f8)
    bias = np.ascontiguousarray(b.astype(np.float32).reshape(64, 1))
    eye = np.eye(64, dtype=np.float32).astype(ml_dtypes.bfloat16)
    ones2 = np.ones((128, 2, 1), np.float32).astype(f8)
    cneg = np.ascontiguousarray(-centers.astype(np.float32).T)  # [K, D]
    common = {"wt": wt, "bias": bias, "eye": eye, "ones2": ones2,
              "cneg": cneg}
    in_maps = []
    for core in range(N_CORES):
        dc = d16[B_PER * core:B_PER * (core + 1)]        # [2, D, N] fp8
        # da[i, s, p, t, j] = desc[i, 128t+p, 1024s+j]
        da = dc.reshape(B_PER, DT, 128, S, NSTR).transpose(0, 3, 2, 1, 4)
        # dt[i, s, p, c, d] = desc[i, d, 1024s+128c+p]
        dt_ = dc.reshape(B_PER, D, S, CPS, 128).transpose(0, 2, 4, 3, 1)
        m = dict(common)
        m["da"] = np.ascontiguousarray(da)
        m["dt"] = np.ascontiguousarray(dt_)
        in_maps.append(m)
    return in_maps


def _run(inputs, trace=False):
    from concourse.bass_utils import run_bass_kernel_spmd

    descriptors = np.asarray(inputs["descriptors"])
    W = np.asarray(inputs["W"])
    b = np.asarray(inputs["b"])
    centers = np.asarray(inputs["centers"])
    nc = _get_nc()
    in_maps = _host_inputs(descriptors, W, b, centers)
    res = run_bass_kernel_spmd(nc, in_maps, list(range(N_CORES)), trace=trace)
    outs = []
    for core in range(N_CORES):
        o = res.results[core]["out"]          # [B_PER, K, D]
        outs.append(np.transpose(o, (0, 2, 1)).reshape(B_PER, D * K))
    full = np.concatenate(outs, axis=0).astype(np.float32)
    return full, res


def kernel(**inputs):
    out, _ = _run(inputs, trace=False)
    return out


if __name__ == "__main__":
    rng = np.random.default_rng(0)
    inputs = {
        "descriptors": rng.standard_normal((B, D, N), dtype=np.float32),
        "W": (rng.standard_normal((K, D)) * 0.05).astype(np.float32),
        "b": (rng.standard_normal((K,)) * 0.05).astype(np.float32),
        "centers": rng.standard_normal((D, K)).astype(np.float32),
    }
    out = kernel(**inputs)
    print("out shape:", out.shape, out.dtype)


# revision 17
# speedup vs baseline: 1.0517x; 1.0517x over previous
"""NetVLAD layer on 8 Trainium2 NeuronCores (Bass/Tile), v4.

Problem: descriptors [B=16, D=512, N=4096] f32, W [K=64, D], b [K],
centers [D, K].
  scores = softmax_K(W @ desc + b)            [B, K, N]
  agg[b,d,k] = sum_n scores[b,k,n] desc[b,d,n]
  vlad = agg - centers * sum_n(scores);  intra-L2-norm over D; global L2.

Sharding: data-parallel over B across 8 cores (2 items per core);
W/b/centers replicated.

v4 design (v1 121.6us -> v2 113.5 -> v3 103.6us):
  v3 was dependency/latency bound: per 4-chunk group PE stalled on the
  softmax chain, ACT paid a separate accumulator-readout per chunk, and
  Exp<->Sqrt activation-table reloads cost 1.3us each.  v4 restructures
  for throughput:
  - desc pre-cast fp8e4m3, both layouts, as in v3 (8.4 MB/core DMA).
  - mm1 keeps W stationary (tiny LDWEIGHTS) and streams desc via fp8
    DoubleRow, producing scores [2x64k, 512n] packed on 128 partitions
    (half 0 at partitions 0-63 via quadrant (0,0), half 1 at 64-127 via
    quadrant (0,64)).  One ACT exp per strip with the natural
    per-partition bias ([b;b]), full 128 lanes.
  - exp chunks are PE-transposed (bf16, identity quadrants) into a
    [128, 8, K] PSUM group; softmax is 3 DVE ops per strip: 3D reduce
    -> Z, reciprocal, and one broadcast multiply -> softT fp8.
  - mm2: agg[K, D] via 4 fp8 DoubleRow matmuls per strip (2 n-chunks
    each); ssum via DoubleRow ones-column-sums, both items packed in
    one PSUM bank at partition offsets 0/64.
  - the two batch items are interleaved strip-by-strip, transposes run
    one stage behind mm1 and mm2 two stages behind, so PE always has
    independent work while a strip's softmax chain completes.
  - global L2 norm folded to the exact constant 1/sqrt(K)=0.125 (after
    intra-norm every k-column has unit norm), so the tail needs a
    single Sqrt per item and only one activation-table switch, after
    all exps are done.
"""

import sys

sys.path.insert(0, "/opt/trn_rl_repo")

import numpy as np
import ml_dtypes

B, D, K, N = 16, 512, 64, 4096
N_CORES = 8
B_PER = B // N_CORES           # 2 items per core
DT = D // 128                  # 4 d-tiles
S = 4                          # n-strips per item
NSTR = N // S                  # 1024 columns per strip
CPS = NSTR // 128              # 8 n-chunks of 128 per strip

_CACHE = {}


def _build():
    import concourse.bass as bass  # noqa: F401
    import concourse.tile as tile
    from concourse import bacc, mybir
    from contextlib import ExitStack

    bf16 = mybir.dt.bfloat16
    f8 = mybir.dt.float8e4
    f32 = mybir.dt.float32
    AF = mybir.ActivationFunctionType
    OP = mybir.AluOpType
    AX = mybir.AxisListType
    DR = mybir.MatmulPerfMode.DoubleRow

    nc = bacc.Bacc("TRN2", target_bir_lowering=False, debug=False,
                   num_devices=N_CORES)

    # desc strips, [item, strip, partition, ...]: one 4 KB row per partition
    da_d = nc.dram_tensor("da", [B_PER, S, 128, DT, NSTR], f8,
                          kind="ExternalInput").ap()
    dt_d = nc.dram_tensor("dt", [B_PER, S, 128, CPS, 512], f8,
                          kind="ExternalInput").ap()
    wt_d = nc.dram_tensor("wt", [DT, 128, K], f8, kind="ExternalInput").ap()
    b_d = nc.dram_tensor("bias", [64, 1], f32, kind="ExternalInput").ap()
    eye_d = nc.dram_tensor("eye", [64, 64], bf16,
                           kind="ExternalInput").ap()
    ones2_d = nc.dram_tensor("ones2", [128, 2, 1], f8,
                             kind="ExternalInput").ap()
    cneg_d = nc.dram_tensor("cneg", [K, D], f32, kind="ExternalInput").ap()
    out_d = nc.dram_tensor("out", [B_PER, K, D], f32,
                           kind="ExternalOutput").ap()

    with tile.TileContext(nc) as tc, ExitStack() as ctx:
        const = ctx.enter_context(tc.tile_pool(name="const", bufs=1))
        sdesc = ctx.enter_context(tc.tile_pool(name="sdesc", bufs=3))
        sdt = ctx.enter_context(tc.tile_pool(name="sdt", bufs=4))
        pexp = ctx.enter_context(tc.tile_pool(name="pexp", bufs=3))
        psoft = ctx.enter_context(tc.tile_pool(name="psoft", bufs=3))
        small = ctx.enter_context(tc.tile_pool(name="small", bufs=16))
        med = ctx.enter_context(tc.tile_pool(name="med", bufs=2))
        # PSUM bank budget (8): scA 1 + scB 1 + xt 2 + agg 2 + ssA 1 + ssB 1
        ps_sc = ctx.enter_context(tc.tile_pool(name="ps_sc", bufs=1,
                                               space="PSUM"))
        ps_xt = ctx.enter_context(tc.tile_pool(name="ps_xt", bufs=2,
                                               space="PSUM"))
        ps_agg = ctx.enter_context(tc.tile_pool(name="ps_agg", bufs=2,
                                                space="PSUM"))
        ps_ss = ctx.enter_context(tc.tile_pool(name="ps_ss", bufs=2,
                                               space="PSUM"))

        # ---- constants (scalar HWDGE queue) ----
        wt_sb = const.tile([128, DT, K], f8, tag="wt")
        for t in range(DT):
            nc.scalar.dma_start(out=wt_sb[:, t, :], in_=wt_d[t])
        b_sb = const.tile([64, 1], f32, tag="b")
        nc.scalar.dma_start(out=b_sb[:], in_=b_d[:])
        eye_sb = const.tile([64, 64], bf16, tag="eye")
        nc.scalar.dma_start(out=eye_sb[:], in_=eye_d[:])
        ones2_sb = const.tile([128, 2, 1], f8, tag="ones2")
        nc.scalar.dma_start(out=ones2_sb[:], in_=ones2_d[:])
        cneg_sb = const.tile([K, D], f32, tag="cneg")
        nc.scalar.dma_start(out=cneg_sb[:], in_=cneg_d[:])
        eps_sb = const.tile([K, 1], f32, tag="eps")
        nc.vector.memset(eps_sb[:], 1e-24)

        agg_tiles = [ps_agg.tile([K, D], f32, tag="agg", name=f"agg{i}")
                     for i in range(B_PER)]
        ss_tiles = [ps_ss.tile([K, 1], f32, tag="ss", name=f"ss{i}")
                    for i in range(B_PER)]

        pend_tr = []   # (i, s, exp_s, dTt) awaiting transpose+softmax
        pend_mm2 = []  # (i, s, soft_g, dTt) awaiting mm2

        def emit_tr(grp):
            i, s, exps, dTt = grp
            xt = ps_xt.tile([128, CPS, K], bf16, tag="xt",
                            name=f"xt{i}{s}")
            for c in range(CPS):
                h, cc = divmod(c, 4)
                nc.tensor.transpose(
                    xt[:, c, :],
                    exps[h][:, 128 * cc:128 * (cc + 1)],
                    eye_sb[:],
                )
            z8 = small.tile([128, CPS], f32, tag="z", name=f"z{i}{s}")
            nc.vector.reduce_sum(z8[:], xt[:], axis=AX.X)
            r8 = small.tile([128, CPS], f32, tag="r", name=f"r{i}{s}")
            nc.vector.reciprocal(r8[:], z8[:])
            soft_g = psoft.tile([128, CPS, K], f8, tag="soft",
                                name=f"soft{i}{s}")
            nc.vector.tensor_mul(soft_g[:], xt[:],
                                 r8[:, :, None].broadcast_to((128, CPS, K)))
            pend_mm2.append((i, s, soft_g, dTt))

        def emit_mm2(grp):
            i, s, soft_g, dTt = grp
            for j in range(CPS // 2):
                nc.tensor.matmul(
                    agg_tiles[i][:], lhsT=soft_g[:, 2 * j:2 * j + 2, :],
                    rhs=dTt[:, 2 * j:2 * j + 2, :], perf_mode=DR,
                    start=(s == 0 and j == 0),
                    stop=(s == S - 1 and j == CPS // 2 - 1))
            for j in range(CPS // 2):
                nc.tensor.matmul(
                    ss_tiles[i][:], lhsT=soft_g[:, 2 * j:2 * j + 2, :],
                    rhs=ones2_sb[:], perf_mode=DR,
                    start=(s == 0 and j == 0),
                    stop=(s == S - 1 and j == CPS // 2 - 1))

        for s in range(S):
            for i in range(B_PER):
                dbf = sdesc.tile([128, DT, NSTR], f8, tag="dbf",
                                 name=f"dbf{i}{s}")
                nc.sync.dma_start(out=dbf[:], in_=da_d[i, s])
                dTt = sdt.tile([128, CPS, 512], f8, tag="dT",
                               name=f"dT{i}{s}")
                nc.scalar.dma_start(out=dTt[:], in_=dt_d[i, s])
                # mm1: scores halves [64k, 512n] via fp8 DoubleRow,
                # W stationary; exp(scores + b) -> bf16 per half
                exps = []
                for h in range(2):
                    scp = ps_sc.tile([64, 512], f32, tag=f"sc{h}",
                                     name=f"sc{h}_{i}{s}")
                    for T in range(2):
                        nc.tensor.matmul(
                            scp[:],
                            lhsT=wt_sb[:, 2 * T:2 * T + 2, :],
                            rhs=dbf[:, 2 * T:2 * T + 2,
                                    512 * h:512 * (h + 1)],
                            perf_mode=DR, start=(T == 0), stop=(T == 1))
                    exp_h = pexp.tile([64, 512], bf16, tag=f"exps{h}",
                                      name=f"exps{h}_{i}{s}")
                    nc.scalar.activation(out=exp_h[:], in_=scp[:],
                                         func=AF.Exp, bias=b_sb[:],
                                         scale=1.0)
                    exps.append(exp_h)
                pend_tr.append((i, s, exps, dTt))
                # software pipeline: transposes 1 stage behind, mm2 2 behind
                if len(pend_tr) > 1:
                    emit_tr(pend_tr.pop(0))
                if len(pend_mm2) > 1:
                    emit_mm2(pend_mm2.pop(0))
        def emit_tail(i):
            ssum_sb = small.tile([K, 1], f32, tag="ssum", name=f"ssum{i}")
            nc.scalar.copy(ssum_sb[:], ss_tiles[i][:])
            vlad_sb = med.tile([K, D], f32, tag="vlad", name=f"vlad{i}")
            nc.vector.scalar_tensor_tensor(
                vlad_sb[:], in0=cneg_sb[:], scalar=ssum_sb[:],
                in1=agg_tiles[i][:], op0=OP.mult, op1=OP.add,
            )
            # row sumsq via ACT Square+accum; rn = exp(-0.5 ln ss)
            sq_sb = med.tile([K, D], f32, tag="sq", name=f"sq{i}")
            ss_sb = small.tile([K, 1], f32, tag="ss2", name=f"ss2{i}")
            nc.scalar.activation(sq_sb[:], vlad_sb[:], func=AF.Square,
                                 accum_out=ss_sb[:])
            ln_sb = small.tile([K, 1], f32, tag="ln", name=f"ln{i}")
            nc.scalar.activation(ln_sb[:], ss_sb[:], func=AF.Ln)
            rn_sb = small.tile([K, 1], f32, tag="rn", name=f"rn{i}")
            nc.scalar.activation(rn_sb[:], ln_sb[:], func=AF.Exp,
                                 scale=-0.5)
            # intra-norm by rn; global norm is exactly 1/sqrt(K) = 0.125
            outT_sb = med.tile([K, D], f32, tag="outT", name=f"outT{i}")
            nc.vector.tensor_scalar(out=outT_sb[:], in0=vlad_sb[:],
                                    scalar1=rn_sb[:], scalar2=0.125,
                                    op0=OP.mult, op1=OP.mult)
            nc.sync.dma_start(out=out_d[i], in_=outT_sb[:])

        while pend_tr:
            emit_tr(pend_tr.pop(0))
        while pend_mm2:
            grp = pend_mm2.pop(0)
            emit_mm2(grp)
            emit_tail(grp[0])

    nc.compile()
    return nc


def _get_nc():
    if "nc" not in _CACHE:
        _CACHE["nc"] = _build()
    return _CACHE["nc"]


def _host_inputs(descriptors, W, b, centers):
    f8 = ml_dtypes.float8_e4m3fn
    d16 = np.asarray(descriptors, dtype=np.float32).astype(f8)  # [B, D, N]
    wt = np.ascontiguousarray(
        W.astype(np.float32).T.reshape(DT, 128, K)).astype(f8)
    bias = np.ascontiguousarray(b.astype(np.float32).reshape(64, 1))
    eye = np.eye(64, dtype=np.float32).astype(ml_dtypes.bfloat16)
    ones2 = np.ones((128, 2, 1), np.float32).astype(f8)
    cneg = np.ascontiguousarray(-centers.astype(np.float32).T)  # [K, D]
    common = {"wt": wt, "bias": bias, "eye": eye, "ones2": ones2,
              "cneg": cneg}
    in_maps = []
    for core in range(N_CORES):
        dc = d16[B_PER * core:B_PER * (core + 1)]        # [2, D, N] fp8
        # da[i, s, p, t, j] = desc[i, 128t+p, 1024s+j]
        da = dc.reshape(B_PER, DT, 128, S, NSTR).transpose(0, 3, 2, 1, 4)
        # dt[i, s, p, c, d] = desc[i, d, 1024s+128c+p]
        dt_ = dc.reshape(B_PER, D, S, CPS, 128).transpose(0, 2, 4, 3, 1)
        m = dict(common)
        m["da"] = np.ascontiguousarray(da)
        m["dt"] = np.ascontiguousarray(dt_)
        in_maps.append(m)
    return in_maps


def _run(inputs, trace=False):
    from concourse.bass_utils import run_bass_kernel_spmd

    descriptors = np.asarray(inputs["descriptors"])
    W = np.asarray(inputs["W"])
    b = np.asarray(inputs["b"])
    centers = np.asarray(inputs["centers"])
    nc = _get_nc()
    in_maps = _host_inputs(descriptors, W, b, centers)
    res = run_bass_kernel_spmd(nc, in_maps, list(range(N_CORES)), trace=trace)
    outs = []
    for core in range(N_CORES):
        o = res.results[core]["out"]          # [B_PER, K, D]
        outs.append(np.transpose(o, (0, 2, 1)).reshape(B_PER, D * K))
    full = np.concatenate(outs, axis=0).astype(np.float32)
    return full, res


def kernel(**inputs):
    out, _ = _run(inputs, trace=False)
    return out


if __name__ == "__main__":
    rng = np.random.default_rng(0)
    inputs = {
        "descriptors": rng.standard_normal((B, D, N), dtype=np.float32),
        "W": (rng.standard_normal((K, D)) * 0.05).astype(np.float32),
        "b": (rng.standard_normal((K,)) * 0.05).astype(np.float32),
        "centers": rng.standard_normal((D, K)).astype(np.float32),
    }
    out = kernel(**inputs)
    print("out shape:", out.shape, out.dtype)
